# revision 25
# baseline (speedup 1.0000x reference)
"""Trainium2 Bass kernel for nn_Node_attention_layer (ragged_sequence).

Full-input contract: kernel(**inputs) takes the unsharded inputs and returns
(out [B,S,HID] f32, k_scores [B,S] f32), matching the reference.

Sharding: data-parallel over batch B=16 across 8 NeuronCores (2 samples per
core); Linear weights replicated; emb_table rows gathered host-side (only
K=16 rows per sample are used).

Per-core device program (SPMD, identical program, per-core data):
  proj   = tanh(x @ W_att + b_att)                    [N=64, D=512]
  projW  = proj @ W_hid[D:] + b_hid                   [64, 512]
           (reassociation: dot_x @ Wh2 == scores @ (proj @ Wh2); softmax rows
            sum to 1 so the +b_hid fold is exact)
  per macro tile of 512 s rows:
    lgT = [projT | kembT].T @ encT + ones80 (x) madd  [80, 512]
          (the mask bias rides the matmul accumulation; -1e18 absorbs the
           row's logits in f32 psum, reproducing masked_fill exactly)
    PE-transpose back to [128, 80] tiles; softmax over 64 / over 16
    (pair-batched vector ops); k_scores = 1/sum(exp(klog - kmax))
    out = tanh(encT.T @ Wh1 + scoresT.T @ projW)

Matmul operands are bf16 (host-converted; softmax/psum math stays f32);
enc/out use host-packed per-macro layouts so every DMA moves >=4KB
contiguous per partition.
"""

import sys

if "/opt/trn_rl_repo" not in sys.path:
    sys.path.insert(0, "/opt/trn_rl_repo")

import numpy as np

B, S, D, N, K, V = 16, 2048, 512, 64, 16, 32000
HID = 512
NEG = -1e18
N_CORES = 8
BPC = B // N_CORES  # samples per core
S_MACRO = 512       # s columns per enc staging tile
S_SUB = 128         # s rows per compute tile
NMT = S // S_MACRO      # 4 macro tiles per sample
NST = S_MACRO // S_SUB  # 4 sub tiles per macro
NK = N + K              # 80

OUT_BF16 = True     # stage the tanh output as bf16 (halves write traffic)

TRACE = False       # test.py sets True to collect exec_time_ns
LAST_RESULT = {}    # test.py reads exec_time_ns etc from here

_CACHE = {}


def _install_ntff_shim():
    """Provide antenv.axon_hooks (missing in this image) so that
    run_bass_kernel_spmd(trace=True) can collect NTFF profiles via the
    axon PJRT .so's C ABI."""
    import types
    import ctypes
    import contextlib

    if "antenv.axon_hooks" in sys.modules:
        return
    mod = types.ModuleType("antenv.axon_hooks")
    state = {"hook": None}

    def set_axon_ntff_profile_hook(h):
        state["hook"] = h

    def get_axon_ntff_profile_hook():
        return state["hook"]

    mod.set_axon_ntff_profile_hook = set_axon_ntff_profile_hook
    mod.get_axon_ntff_profile_hook = get_axon_ntff_profile_hook
    sys.modules["antenv.axon_hooks"] = mod
    try:
        import antenv

        antenv.axon_hooks = mod
    except ImportError:
        pass

    so_path = "/opt/axon/libaxon_pjrt.so"
    try:
        lib = ctypes.CDLL(so_path)
        if not hasattr(lib, "axon_start_nrt_profile"):
            return
    except OSError:
        return
    lib.axon_start_nrt_profile.argtypes = [
        ctypes.POINTER(ctypes.c_int64), ctypes.c_size_t]
    lib.axon_start_nrt_profile.restype = ctypes.c_int64
    lib.axon_stop_nrt_profile.argtypes = [ctypes.c_char_p]
    lib.axon_stop_nrt_profile.restype = ctypes.c_int64

    @contextlib.contextmanager
    def _hook(output_dir, device_ids):
        import jax

        jax.devices()
        if device_ids:
            ids = (ctypes.c_int64 * len(device_ids))(*device_ids)
            rc = lib.axon_start_nrt_profile(ids, len(device_ids))
        else:
            rc = lib.axon_start_nrt_profile(None, 0)
        if rc != 0:
            raise RuntimeError(f"axon_start_nrt_profile rc={rc}")
        try:
            yield
        finally:
            n = lib.axon_stop_nrt_profile(str(output_dir).encode())
            print(f"ntff profile: {n} file(s) written to {output_dir}",
                  file=sys.stderr)

    set_axon_ntff_profile_hook(_hook)


def _build():
    import concourse.bacc as bacc
    import concourse.mybir as mybir
    import concourse.tile as tile
    from concourse.masks import make_identity
    import concourse.bass as bass

    f32 = mybir.dt.float32
    bf16 = mybir.dt.bfloat16
    out_dt = bf16 if OUT_BF16 else f32
    AF = mybir.ActivationFunctionType
    AX = mybir.AxisListType
    OP = mybir.AluOpType

    nc = bacc.Bacc("TRN2", target_bir_lowering=False, debug=False,
                   num_devices=N_CORES)

    # enc host-packed bf16: [b, mt, p, c, s_loc] so each macro load is one
    # DMA with 4KB contiguous per partition
    encN_d = nc.dram_tensor("encN", [BPC, NMT, 128, 4, S_MACRO], bf16,
                            kind="ExternalInput").ap()
    # misc: per-partition packed [xT_b0 | xT_b1 | kemb_b0 | kemb_b1]
    misc_d = nc.dram_tensor("misc", [128, 2 * 8 * N + 2 * 4 * K], bf16,
                            kind="ExternalInput").ap()
    # rowb: [madd_b0 | madd_b1 | batt] on one partition row
    rowb_d = nc.dram_tensor("rowb", [1, 2 * S + D], bf16,
                            kind="ExternalInput").ap()
    Wa_d = nc.dram_tensor("Wa", [128, 8, 512], bf16, kind="ExternalInput").ap()
    Wh_d = nc.dram_tensor("Wh", [128, 8, 512], bf16, kind="ExternalInput").ap()
    bhid_d = nc.dram_tensor("bhid", [1, HID], f32, kind="ExternalInput").ap()
    # out host-packed: [b, mt, p, st, h]; host unscrambles
    out_d = nc.dram_tensor("out", [BPC, NMT, 128, NST, HID], out_dt,
                           kind="ExternalOutput").ap()
    ks_d = nc.dram_tensor("ks", [BPC, S], f32, kind="ExternalOutput").ap()

    ks_v = ks_d.rearrange("b (j p) -> b j p", p=128)             # [2,16,128]

    with tile.TileContext(nc) as tc:
        with tc.tile_pool(name="consts", bufs=1) as consts, \
             tc.tile_pool(name="samp", bufs=2) as samp, \
             tc.tile_pool(name="encp", bufs=8) as encp, \
             tc.tile_pool(name="work", bufs=4) as work, \
             tc.tile_pool(name="outp", bufs=3) as outp, \
             tc.tile_pool(name="pslogT", bufs=1, space="PSUM") as pslogT, \
             tc.tile_pool(name="pslg", bufs=2, space="PSUM") as pslg, \
             tc.tile_pool(name="psout", bufs=3, space="PSUM") as psout, \
             tc.tile_pool(name="pst", bufs=2, space="PSUM") as pst:

            ident = consts.tile([128, 128], f32)
            make_identity(nc, ident)
            ident_bf = consts.tile([128, 128], bf16)
            nc.vector.tensor_copy(out=ident_bf, in_=ident)
            ones_bf = consts.tile([1, NK], bf16)
            nc.vector.memset(ones_bf, 1.0)

            # HAM warm-up: ~4us of dense matmuls while the first DMAs
            # stream, so the PE clock is at 2.4GHz when real work arrives
            warm_ps = pst.tile([128, 128], f32, tag="tp")
            for i in range(40):
                nc.tensor.matmul(warm_ps, ident_bf, ident_bf,
                                 start=(i == 0), stop=(i == 39))

            # DMA issue order tuned for the startup critical path:
            # misc (tiny, feeds proj) -> Wa (feeds proj) -> first enc macro
            # -> Wh -> second enc macro
            misc_t = consts.tile([128, 2 * 8 * N + 2 * 4 * K], bf16)
            nc.sync.dma_start(out=misc_t, in_=misc_d)
            Wa_t = consts.tile([128, 8, 512], bf16)
            nc.sync.dma_start(out=Wa_t, in_=Wa_d)
            enc_tiles = {}
            for b, mt in [(0, 0)]:
                enc_t = encp.tile([128, 4, S_MACRO], bf16, tag="enc_t")
                nc.sync.dma_start(out=enc_t, in_=encN_d[b, mt])
                enc_tiles[(b, mt)] = enc_t
            Wh_t = consts.tile([128, 8, 512], bf16)
            nc.sync.dma_start(out=Wh_t, in_=Wh_d)
            for b, mt in [(0, 1)]:
                enc_t = encp.tile([128, 4, S_MACRO], bf16, tag="enc_t")
                nc.sync.dma_start(out=enc_t, in_=encN_d[b, mt])
                enc_tiles[(b, mt)] = enc_t
            rowb_t = consts.tile([1, 2 * S + D], bf16)
            nc.sync.dma_start(out=rowb_t, in_=rowb_d)
            batt_t = rowb_t[:, 2 * S:2 * S + D]
            # b_hid broadcast to 64 partitions (stride-0 partition dim)
            bhid_bc = consts.tile([N, HID], f32)
            bhid_bcast_ap = bass.AP(
                tensor=bhid_d.tensor, offset=bhid_d.offset,
                ap=[[0, N], bhid_d.ap[1]],
            )
            nc.sync.dma_start(out=bhid_bc, in_=bhid_bcast_ap)

            pkT_b = []
            projW_b = []
            maddT_b = []
            kst_b = []
            for b in range(BPC):
                xT_t = misc_t[:, b * 8 * N:(b + 1) * 8 * N].rearrange(
                    "p (c n) -> p c n", c=8)
                kemb_t = misc_t[:, 2 * 8 * N + b * 4 * K:
                                2 * 8 * N + (b + 1) * 4 * K].rearrange(
                    "p (c k) -> p c k", c=4)
                # madd as a free-dim row vector for the PE mask-bias matmul
                maddT = rowb_t[:, b * S:(b + 1) * S]
                maddT_b.append(maddT)

                # proj = tanh(x @ Wa + b_att)  [64, 512]
                proj_ps = pst.tile([N, D], f32, tag="tp")
                for c in range(8):
                    nc.tensor.matmul(proj_ps, xT_t[:, c, :], Wa_t[:, c, :],
                                     start=(c == 0), stop=False)
                nc.tensor.matmul(proj_ps, ones_bf[:, 0:N], batt_t,
                                 start=False, stop=True)
                proj_f = samp.tile([N, D], f32)
                nc.scalar.activation(out=proj_f, in_=proj_ps, func=AF.Tanh)

                # pkT[:, c, 0:64] = projT chunk c;  pkT[:, c, 64:80] = kembT
                pkT = samp.tile([128, 4, NK], bf16)
                for c in range(4):
                    pt_ps = pst.tile([128, N], f32, tag="tp")
                    nc.tensor.transpose(
                        pt_ps, proj_f[:, c * 128:(c + 1) * 128],
                        ident[0:N, 0:N])
                    nc.vector.tensor_copy(out=pkT[:, c, 0:N], in_=pt_ps)
                nc.vector.tensor_copy(out=pkT[:, :, N:NK], in_=kemb_t)
                pkT_b.append(pkT)

                # projW = proj @ Wh2 + b_hid  [64, 512]
                pw_ps = pst.tile([N, HID], f32, tag="tp")
                for c in range(4):
                    nc.tensor.matmul(pw_ps, pkT[:, c, 0:N], Wh_t[:, 4 + c, :],
                                     start=(c == 0), stop=(c == 3))
                # projW duplicated to both partition halves so the scores
                # matmul can use lhsT slices at base partition 0 or 64
                projW_t = samp.tile([2 * N, HID], bf16)
                nc.vector.tensor_add(projW_t[0:N, :], pw_ps, bhid_bc)
                nc.vector.tensor_add(projW_t[N:2 * N, :], pw_ps, bhid_bc)
                projW_b.append(projW_t)

                kst_sb = samp.tile([128, 16], f32, tag="kst")
                kst_b.append(kst_sb)

            for b in range(BPC):
                for mt in range(NMT):
                    if (b, mt) in enc_tiles:
                        continue
                    enc_t = encp.tile([128, 4, S_MACRO], bf16, tag="enc_t")
                    nc.sync.dma_start(out=enc_t, in_=encN_d[b, mt])
                    enc_tiles[(b, mt)] = enc_t

            for b in range(BPC):
                pkT = pkT_b[b]
                projW_t = projW_b[b]
                maddT = maddT_b[b]
                kst_sb = kst_b[b]
                for mt in range(NMT):
                    enc_t = enc_tiles[(b, mt)]

                    # (1) transposed: lgT = [projT|kembT].T @ enc + mask
                    # bias (ones80 outer madd_row rides the accumulation)
                    lgT_ps = pslogT.tile([NK, S_MACRO], f32)
                    nc.tensor.matmul(
                        lgT_ps, ones_bf,
                        maddT[:, mt * S_MACRO:(mt + 1) * S_MACRO],
                        start=True, stop=False)
                    for c in range(4):
                        nc.tensor.matmul(lgT_ps, pkT[:, c, :], enc_t[:, c, :],
                                         start=False, stop=(c == 3))
                    lgT_sb = work.tile([NK, S_MACRO], f32)
                    nc.vector.tensor_copy(out=lgT_sb, in_=lgT_ps)

                    o4_sb = outp.tile([128, NST, HID], out_dt)
                    out_v = out_d[b, mt].rearrange("p (q a) h -> p q a h", q=2)

                    for pr in range(NST // 2):
                        j0 = mt * NST + 2 * pr
                        # transpose two subtiles into one [128, 2, 80] psum
                        lg_ps = pslg.tile([128, 2, NK], f32)
                        for i in range(2):
                            c0 = (2 * pr + i) * S_SUB
                            nc.tensor.matmul(
                                lg_ps[:, i, :], lgT_sb[:, c0:c0 + S_SUB],
                                ident[0:NK, 0:NK], is_transpose=True,
                                start=True, stop=True)

                        # softmax pieces, pair-batched, reading psum direct
                        nmax = work.tile([128, 2], f32)
                        nc.vector.tensor_reduce(
                            out=nmax, in_=lg_ps[:, :, 0:N], axis=AX.X,
                            op=OP.max, negate=True)
                        kmax = work.tile([128, 2], f32)
                        nc.vector.tensor_reduce(
                            out=kmax, in_=lg_ps[:, :, N:NK], axis=AX.X,
                            op=OP.max, negate=True)
                        e_in = work.tile([128, 2, NK], f32)
                        nc.vector.tensor_add(
                            e_in[:, :, 0:N], lg_ps[:, :, 0:N],
                            nmax.broadcast_to([128, 2, N]))
                        nc.vector.tensor_add(
                            e_in[:, :, N:NK], lg_ps[:, :, N:NK],
                            kmax.broadcast_to([128, 2, K]))
                        e_out = work.tile([128, 2, NK], f32)
                        nc.scalar.activation(out=e_out, in_=e_in, func=AF.Exp)
                        nsum = work.tile([128, 2], f32)
                        nc.vector.tensor_reduce(
                            out=nsum, in_=e_out[:, :, 0:N], axis=AX.X,
                            op=OP.add)
                        ksum = work.tile([128, 2], f32)
                        nc.vector.tensor_reduce(
                            out=ksum, in_=e_out[:, :, N:NK], axis=AX.X,
                            op=OP.add)
                        rn = work.tile([128, 2], f32)
                        nc.vector.reciprocal(out=rn, in_=nsum)
                        nc.vector.reciprocal(out=kst_sb[:, j0:j0 + 2],
                                             in_=ksum)
                        ps_sc = work.tile([128, 2, N], f32)
                        nc.vector.tensor_mul(
                            ps_sc, e_out[:, :, 0:N],
                            rn.broadcast_to([128, 2, N]))

                        # one [128,128] transpose covers both subtiles:
                        # rows 0:64 = scoresT of subtile 2pr, 64:128 = 2pr+1
                        scT_ps = pst.tile([128, 128], f32, tag="tp")
                        nc.tensor.transpose(
                            scT_ps, ps_sc.rearrange("p a n -> p (a n)"),
                            ident)
                        scT_sb = work.tile([128, 128], bf16)
                        nc.vector.tensor_copy(out=scT_sb, in_=scT_ps)

                        for i in range(2):
                            st = 2 * pr + i
                            sl = slice(st * S_SUB, (st + 1) * S_SUB)
                            o_ps = psout.tile([128, HID], f32)
                            for c in range(4):
                                nc.tensor.matmul(o_ps, enc_t[:, c, sl],
                                                 Wh_t[:, c, :],
                                                 start=(c == 0), stop=False)
                            nc.tensor.matmul(
                                o_ps, scT_sb[i * N:(i + 1) * N, :],
                                projW_t[i * N:(i + 1) * N, :],
                                start=False, stop=True)
                            nc.scalar.activation(out=o4_sb[:, st, :],
                                                 in_=o_ps, func=AF.Tanh)
                        nc.sync.dma_start(
                            out=out_v[:, pr],
                            in_=o4_sb[:, 2 * pr:2 * pr + 2, :])



                kT_ps = pst.tile([16, 128], f32, tag="tp")
                nc.tensor.transpose(kT_ps, kst_sb, ident)
                kT_sb = samp.tile([16, 128], f32)
                nc.vector.tensor_copy(out=kT_sb, in_=kT_ps)
                nc.sync.dma_start(out=ks_v[b], in_=kT_sb)

    nc.compile()
    return nc


def _get_nc():
    if "nc" not in _CACHE:
        _CACHE["nc"] = _build()
    return _CACHE["nc"]


def kernel(enc_outputs, x, key_concepts, mask_enc, W_att, b_att, W_hid, b_hid,
           emb_table):
    _install_ntff_shim()
    import ml_dtypes
    from concourse.bass_utils import run_bass_kernel_spmd

    bf = ml_dtypes.bfloat16
    enc_outputs = np.asarray(enc_outputs, dtype=np.float32)
    x = np.asarray(x, dtype=np.float32)
    key_concepts = np.asarray(key_concepts)
    mask_enc = np.asarray(mask_enc)
    W_att = np.asarray(W_att, dtype=np.float32)
    b_att = np.asarray(b_att, dtype=np.float32)
    W_hid = np.asarray(W_hid, dtype=np.float32)
    b_hid = np.asarray(b_hid, dtype=np.float32)
    emb_table = np.asarray(emb_table, dtype=np.float32)

    scale = np.float32(D ** -0.5)
    # host prep: packed enc layout [b, mt, p, c, s_loc], gather, mask encoding
    # encN[b, mt, p, c, s] = enc[b, mt*512 + s, c*128 + p]
    encN = np.ascontiguousarray(
        enc_outputs.reshape(B, NMT, S_MACRO, 4, 128).transpose(0, 1, 4, 3, 2)
    ).astype(bf)
    # xTp[b, p, c, n] = x[b, n, c*128+p]
    xTp = x.reshape(B, N, 8, 128).transpose(0, 3, 2, 1).astype(bf)
    k_emb = emb_table[key_concepts.astype(np.int64)] * scale      # [B,K,D]
    # kembTp[b, p, c, k] = k_emb[b, k, c*128+p]
    kembTp = k_emb.reshape(B, K, 4, 128).transpose(0, 3, 2, 1).astype(bf)
    madd = np.where(mask_enc, np.float32(NEG), np.float32(0.0)).astype(bf)

    batt = b_att.reshape(1, D).astype(bf)
    bhid = np.ascontiguousarray(b_hid.reshape(1, HID))
    Wa = np.ascontiguousarray(
        W_att.reshape(8, 128, 512).transpose(1, 0, 2)).astype(bf)
    Wh = np.ascontiguousarray(
        W_hid.reshape(8, 128, 512).transpose(1, 0, 2)).astype(bf)

    in_maps = []
    for i in range(N_CORES):
        lo, hi = i * BPC, (i + 1) * BPC
        misc = np.concatenate(
            [xTp[lo + b].reshape(128, 8 * N) for b in range(BPC)]
            + [kembTp[lo + b].reshape(128, 4 * K) for b in range(BPC)],
            axis=1)
        rowb = np.concatenate(
            [madd[lo + b].reshape(1, S) for b in range(BPC)] + [batt],
            axis=1)
        in_maps.append({
            "encN": np.ascontiguousarray(encN[lo:hi]),
            "misc": np.ascontiguousarray(misc),
            "rowb": np.ascontiguousarray(rowb),
            "Wa": Wa,
            "Wh": Wh,
            "bhid": bhid,
        })

    nc = _get_nc()
    res = run_bass_kernel_spmd(nc, in_maps, core_ids=list(range(N_CORES)),
                               trace=TRACE)
    LAST_RESULT["exec_time_ns"] = res.exec_time_ns
    LAST_RESULT["mean_exec_time_ns"] = res.mean_exec_time_ns

    # out device layout [BPC, mt, p, st, h] -> [BPC, S, HID]
    outs = []
    for i in range(N_CORES):
        o = np.asarray(res.results[i]["out"])
        if o.dtype != np.float32:
            o = o.astype(np.float32)
        o = o.reshape(BPC, NMT, 128, NST, HID).transpose(0, 1, 3, 2, 4)
        outs.append(o.reshape(BPC, S, HID))
    out = np.concatenate(outs, axis=0)
    ks = np.concatenate([res.results[i]["ks"] for i in range(N_CORES)], axis=0)
    return out, ks


# revision 26
# speedup vs baseline: 1.1464x; 1.1464x over previous
"""Trainium2 Bass kernel for nn_Node_attention_layer (ragged_sequence).

Full-input contract: kernel(**inputs) takes the unsharded inputs and returns
(out [B,S,HID] f32, k_scores [B,S] f32), matching the reference.

Sharding: data-parallel over batch B=16 across 8 NeuronCores (2 samples per
core); Linear weights replicated; emb_table rows gathered host-side (only
K=16 rows per sample are used).

Per-core device program (SPMD, identical program, per-core data):
  proj   = tanh(x @ W_att + b_att)                    [N=64, D=512]
  projW  = proj @ W_hid[D:] + b_hid                   [64, 512]
           (reassociation: dot_x @ Wh2 == scores @ (proj @ Wh2); softmax rows
            sum to 1 so the +b_hid fold is exact)
  per macro tile of 512 s rows:
    lgT = [projT | kembT].T @ encT + ones80 (x) madd  [80, 512]
          (the mask bias rides the matmul accumulation; -1e18 absorbs the
           row's logits in f32 psum, reproducing masked_fill exactly)
    PE-transpose back to [128, 80] tiles; softmax over 64 / over 16
    (pair-batched vector ops); k_scores = 1/sum(exp(klog - kmax))
    out = tanh(encT.T @ Wh1 + scoresT.T @ projW)

Matmul operands are bf16 (host-converted; softmax/psum math stays f32);
enc/out use host-packed per-macro layouts so every DMA moves >=4KB
contiguous per partition.
"""

import sys

if "/opt/trn_rl_repo" not in sys.path:
    sys.path.insert(0, "/opt/trn_rl_repo")

import numpy as np

B, S, D, N, K, V = 16, 2048, 512, 64, 16, 32000
HID = 512
NEG = -1e18
N_CORES = 8
BPC = B // N_CORES  # samples per core
S_MACRO = 512       # s columns per enc staging tile
S_SUB = 128         # s rows per compute tile
NMT = S // S_MACRO      # 4 macro tiles per sample
NST = S_MACRO // S_SUB  # 4 sub tiles per macro
NK = N + K              # 80

OUT_BF16 = True     # stage the tanh output as bf16 (halves write traffic)

TRACE = False       # test.py sets True to collect exec_time_ns
LAST_RESULT = {}    # test.py reads exec_time_ns etc from here

_CACHE = {}


def _install_ntff_shim():
    """Provide antenv.axon_hooks (missing in this image) so that
    run_bass_kernel_spmd(trace=True) can collect NTFF profiles via the
    axon PJRT .so's C ABI."""
    import types
    import ctypes
    import contextlib

    if "antenv.axon_hooks" in sys.modules:
        return
    mod = types.ModuleType("antenv.axon_hooks")
    state = {"hook": None}

    def set_axon_ntff_profile_hook(h):
        state["hook"] = h

    def get_axon_ntff_profile_hook():
        return state["hook"]

    mod.set_axon_ntff_profile_hook = set_axon_ntff_profile_hook
    mod.get_axon_ntff_profile_hook = get_axon_ntff_profile_hook
    sys.modules["antenv.axon_hooks"] = mod
    try:
        import antenv

        antenv.axon_hooks = mod
    except ImportError:
        pass

    so_path = "/opt/axon/libaxon_pjrt.so"
    try:
        lib = ctypes.CDLL(so_path)
        if not hasattr(lib, "axon_start_nrt_profile"):
            return
    except OSError:
        return
    lib.axon_start_nrt_profile.argtypes = [
        ctypes.POINTER(ctypes.c_int64), ctypes.c_size_t]
    lib.axon_start_nrt_profile.restype = ctypes.c_int64
    lib.axon_stop_nrt_profile.argtypes = [ctypes.c_char_p]
    lib.axon_stop_nrt_profile.restype = ctypes.c_int64

    @contextlib.contextmanager
    def _hook(output_dir, device_ids):
        import jax

        jax.devices()
        if device_ids:
            ids = (ctypes.c_int64 * len(device_ids))(*device_ids)
            rc = lib.axon_start_nrt_profile(ids, len(device_ids))
        else:
            rc = lib.axon_start_nrt_profile(None, 0)
        if rc != 0:
            raise RuntimeError(f"axon_start_nrt_profile rc={rc}")
        try:
            yield
        finally:
            n = lib.axon_stop_nrt_profile(str(output_dir).encode())
            print(f"ntff profile: {n} file(s) written to {output_dir}",
                  file=sys.stderr)

    set_axon_ntff_profile_hook(_hook)


def _build():
    import concourse.bacc as bacc
    import concourse.mybir as mybir
    import concourse.tile as tile
    from concourse.masks import make_identity
    import concourse.bass as bass

    f32 = mybir.dt.float32
    bf16 = mybir.dt.bfloat16
    out_dt = bf16 if OUT_BF16 else f32
    AF = mybir.ActivationFunctionType
    AX = mybir.AxisListType
    OP = mybir.AluOpType

    nc = bacc.Bacc("TRN2", target_bir_lowering=False, debug=False,
                   num_devices=N_CORES)

    # enc host-packed bf16: [b, mt, p, c, s_loc] so each macro load is one
    # DMA with 4KB contiguous per partition
    encN_d = nc.dram_tensor("encN", [BPC, NMT, 128, 4, S_MACRO], bf16,
                            kind="ExternalInput").ap()
    # misc: per-partition packed [xT_b0 | xT_b1 | kemb_b0 | kemb_b1]
    misc_d = nc.dram_tensor("misc", [128, 2 * 8 * N + 2 * 4 * K], bf16,
                            kind="ExternalInput").ap()
    # rowb: [madd_b0 | madd_b1 | batt] on one partition row
    rowb_d = nc.dram_tensor("rowb", [1, 2 * S + D], bf16,
                            kind="ExternalInput").ap()
    Wa_d = nc.dram_tensor("Wa", [128, 8, 512], bf16, kind="ExternalInput").ap()
    Wh_d = nc.dram_tensor("Wh", [128, 8, 512], bf16, kind="ExternalInput").ap()
    bhid_d = nc.dram_tensor("bhid", [1, HID], f32, kind="ExternalInput").ap()
    # out host-packed: [b, mt, p, st, h]; host unscrambles
    out_d = nc.dram_tensor("out", [BPC, NMT, 128, NST, HID], out_dt,
                           kind="ExternalOutput").ap()
    ks_d = nc.dram_tensor("ks", [BPC, S], f32, kind="ExternalOutput").ap()

    ks_v = ks_d.rearrange("b (j p) -> b j p", p=128)             # [2,16,128]

    with tile.TileContext(nc) as tc:
        with tc.tile_pool(name="consts", bufs=1) as consts, \
             tc.tile_pool(name="samp", bufs=2) as samp, \
             tc.tile_pool(name="encp", bufs=6) as encp, \
             tc.tile_pool(name="work", bufs=4) as work, \
             tc.tile_pool(name="outp", bufs=2) as outp, \
             tc.tile_pool(name="pslogT", bufs=1, space="PSUM") as pslogT, \
             tc.tile_pool(name="pslg", bufs=2, space="PSUM") as pslg, \
             tc.tile_pool(name="psout", bufs=3, space="PSUM") as psout, \
             tc.tile_pool(name="pst", bufs=2, space="PSUM") as pst:

            ident = consts.tile([128, 128], f32)
            make_identity(nc, ident)
            ident_bf = consts.tile([128, 128], bf16)
            nc.vector.tensor_copy(out=ident_bf, in_=ident)
            ones_bf = consts.tile([1, NK], bf16)
            nc.vector.memset(ones_bf, 1.0)

            # HAM warm-up: ~4us of dense matmuls while the first DMAs
            # stream, so the PE clock is at 2.4GHz when real work arrives
            warm_ps = pst.tile([128, 128], f32, tag="tp")
            for i in range(40):
                nc.tensor.matmul(warm_ps, ident_bf, ident_bf,
                                 start=(i == 0), stop=(i == 39))

            # DMA issue order tuned for the startup critical path:
            # misc (tiny, feeds proj) -> Wa (feeds proj) -> first enc macro
            # -> Wh -> second enc macro
            misc_t = consts.tile([128, 2 * 8 * N + 2 * 4 * K], bf16)
            nc.sync.dma_start(out=misc_t, in_=misc_d)
            Wa_t = consts.tile([128, 8, 512], bf16)
            nc.sync.dma_start(out=Wa_t, in_=Wa_d)
            enc_tiles = {}
            for b, mt in [(0, 0)]:
                enc_t = encp.tile([128, 4, S_MACRO], bf16, tag="enc_t")
                nc.sync.dma_start(out=enc_t, in_=encN_d[b, mt])
                enc_tiles[(b, mt)] = enc_t
            Wh_t = consts.tile([128, 8, 512], bf16)
            nc.sync.dma_start(out=Wh_t, in_=Wh_d)
            for b, mt in [(0, 1)]:
                enc_t = encp.tile([128, 4, S_MACRO], bf16, tag="enc_t")
                nc.sync.dma_start(out=enc_t, in_=encN_d[b, mt])
                enc_tiles[(b, mt)] = enc_t
            rowb_t = consts.tile([1, 2 * S + D], bf16)
            nc.sync.dma_start(out=rowb_t, in_=rowb_d)
            batt_t = rowb_t[:, 2 * S:2 * S + D]
            # b_hid broadcast to 64 partitions (stride-0 partition dim)
            bhid_bc = consts.tile([N, HID], f32)
            bhid_bcast_ap = bass.AP(
                tensor=bhid_d.tensor, offset=bhid_d.offset,
                ap=[[0, N], bhid_d.ap[1]],
            )
            nc.sync.dma_start(out=bhid_bc, in_=bhid_bcast_ap)

            pkT_b = []
            projW_b = []
            maddT_b = []
            kst_b = []
            for b in range(BPC):
                xT_t = misc_t[:, b * 8 * N:(b + 1) * 8 * N].rearrange(
                    "p (c n) -> p c n", c=8)
                kemb_t = misc_t[:, 2 * 8 * N + b * 4 * K:
                                2 * 8 * N + (b + 1) * 4 * K].rearrange(
                    "p (c k) -> p c k", c=4)
                # madd as a free-dim row vector for the PE mask-bias matmul
                maddT = rowb_t[:, b * S:(b + 1) * S]
                maddT_b.append(maddT)

                # proj = tanh(x @ Wa + b_att)  [64, 512]
                proj_ps = pst.tile([N, D], f32, tag="tp")
                for c in range(8):
                    nc.tensor.matmul(proj_ps, xT_t[:, c, :], Wa_t[:, c, :],
                                     start=(c == 0), stop=False)
                nc.tensor.matmul(proj_ps, ones_bf[:, 0:N], batt_t,
                                 start=False, stop=True)
                proj_f = samp.tile([N, D], f32)
                nc.scalar.activation(out=proj_f, in_=proj_ps, func=AF.Tanh)

                # pkT[:, c, 0:64] = projT chunk c;  pkT[:, c, 64:80] = kembT
                pkT = samp.tile([128, 4, NK], bf16)
                for c in range(4):
                    pt_ps = pst.tile([128, N], f32, tag="tp")
                    nc.tensor.transpose(
                        pt_ps, proj_f[:, c * 128:(c + 1) * 128],
                        ident[0:N, 0:N])
                    nc.vector.tensor_copy(out=pkT[:, c, 0:N], in_=pt_ps)
                nc.vector.tensor_copy(out=pkT[:, :, N:NK], in_=kemb_t)
                pkT_b.append(pkT)

                # projW = proj @ Wh2 + b_hid  [64, 512]
                pw_ps = pst.tile([N, HID], f32, tag="tp")
                for c in range(4):
                    nc.tensor.matmul(pw_ps, pkT[:, c, 0:N], Wh_t[:, 4 + c, :],
                                     start=(c == 0), stop=(c == 3))
                # projW duplicated to both partition halves so the scores
                # matmul can use lhsT slices at base partition 0 or 64
                projW_t = samp.tile([2 * N, HID], bf16)
                nc.vector.tensor_add(projW_t[0:N, :], pw_ps, bhid_bc)
                nc.vector.tensor_add(projW_t[N:2 * N, :], pw_ps, bhid_bc)
                projW_b.append(projW_t)

                kst_sb = samp.tile([128, 16], f32, tag="kst")
                kst_b.append(kst_sb)

            for b in range(BPC):
                for mt in range(NMT):
                    if (b, mt) in enc_tiles:
                        continue
                    enc_t = encp.tile([128, 4, S_MACRO], bf16, tag="enc_t")
                    nc.sync.dma_start(out=enc_t, in_=encN_d[b, mt])
                    enc_tiles[(b, mt)] = enc_t

            for b in range(BPC):
                pkT = pkT_b[b]
                projW_t = projW_b[b]
                maddT = maddT_b[b]
                kst_sb = kst_b[b]
                for mt in range(NMT):
                    enc_t = enc_tiles[(b, mt)]

                    # (1) transposed: lgT = [projT|kembT].T @ enc + mask
                    # bias (ones80 outer madd_row rides the accumulation)
                    lgT_ps = pslogT.tile([NK, S_MACRO], f32)
                    nc.tensor.matmul(
                        lgT_ps, ones_bf,
                        maddT[:, mt * S_MACRO:(mt + 1) * S_MACRO],
                        start=True, stop=False)
                    for c in range(4):
                        nc.tensor.matmul(lgT_ps, pkT[:, c, :], enc_t[:, c, :],
                                         start=False, stop=(c == 3))
                    lgT_sb = work.tile([NK, S_MACRO], f32)
                    nc.vector.tensor_copy(out=lgT_sb, in_=lgT_ps)

                    o4_sb = outp.tile([128, NST, HID], out_dt)
                    out_v = out_d[b, mt].rearrange("p (q a) h -> p q a h", q=2)

                    for pr in range(NST // 2):
                        j0 = mt * NST + 2 * pr
                        # transpose two subtiles into one [128, 2, 80] psum
                        lg_ps = pslg.tile([128, 2, NK], f32)
                        for i in range(2):
                            c0 = (2 * pr + i) * S_SUB
                            nc.tensor.matmul(
                                lg_ps[:, i, :], lgT_sb[:, c0:c0 + S_SUB],
                                ident[0:NK, 0:NK], is_transpose=True,
                                start=True, stop=True)

                        # softmax pieces, pair-batched, reading psum direct
                        nmax = work.tile([128, 2], f32)
                        nc.vector.tensor_reduce(
                            out=nmax, in_=lg_ps[:, :, 0:N], axis=AX.X,
                            op=OP.max, negate=True)
                        kmax = work.tile([128, 2], f32)
                        nc.vector.tensor_reduce(
                            out=kmax, in_=lg_ps[:, :, N:NK], axis=AX.X,
                            op=OP.max, negate=True)
                        e_in = work.tile([128, 2, NK], f32)
                        nc.vector.tensor_add(
                            e_in[:, :, 0:N], lg_ps[:, :, 0:N],
                            nmax.broadcast_to([128, 2, N]))
                        nc.vector.tensor_add(
                            e_in[:, :, N:NK], lg_ps[:, :, N:NK],
                            kmax.broadcast_to([128, 2, K]))
                        e_out = work.tile([128, 2, NK], f32)
                        nc.scalar.activation(out=e_out, in_=e_in, func=AF.Exp)
                        nsum = work.tile([128, 2], f32)
                        nc.vector.tensor_reduce(
                            out=nsum, in_=e_out[:, :, 0:N], axis=AX.X,
                            op=OP.add)
                        ksum = work.tile([128, 2], f32)
                        nc.vector.tensor_reduce(
                            out=ksum, in_=e_out[:, :, N:NK], axis=AX.X,
                            op=OP.add)
                        rn = work.tile([128, 2], f32)
                        nc.vector.reciprocal(out=rn, in_=nsum)
                        nc.vector.reciprocal(out=kst_sb[:, j0:j0 + 2],
                                             in_=ksum)
                        ps_sc = work.tile([128, 2, N], f32)
                        nc.vector.tensor_mul(
                            ps_sc, e_out[:, :, 0:N],
                            rn.broadcast_to([128, 2, N]))

                        # one [128,128] transpose covers both subtiles:
                        # rows 0:64 = scoresT of subtile 2pr, 64:128 = 2pr+1
                        scT_ps = pst.tile([128, 128], f32, tag="tp")
                        nc.tensor.transpose(
                            scT_ps, ps_sc.rearrange("p a n -> p (a n)"),
                            ident)
                        scT_sb = work.tile([128, 128], bf16)
                        nc.vector.tensor_copy(out=scT_sb, in_=scT_ps)

                        for i in range(2):
                            st = 2 * pr + i
                            sl = slice(st * S_SUB, (st + 1) * S_SUB)
                            o_ps = psout.tile([128, HID], f32)
                            for c in range(4):
                                nc.tensor.matmul(o_ps, enc_t[:, c, sl],
                                                 Wh_t[:, c, :],
                                                 start=(c == 0), stop=False)
                            nc.tensor.matmul(
                                o_ps, scT_sb[i * N:(i + 1) * N, :],
                                projW_t[i * N:(i + 1) * N, :],
                                start=False, stop=True)
                            nc.scalar.activation(out=o4_sb[:, st, :],
                                                 in_=o_ps, func=AF.Tanh)
                        nc.sync.dma_start(
                            out=out_v[:, pr],
                            in_=o4_sb[:, 2 * pr:2 * pr + 2, :])



                kT_ps = pst.tile([16, 128], f32, tag="tp")
                nc.tensor.transpose(kT_ps, kst_sb, ident)
                kT_sb = samp.tile([16, 128], f32)
                nc.vector.tensor_copy(out=kT_sb, in_=kT_ps)
                nc.sync.dma_start(out=ks_v[b], in_=kT_sb)

    nc.compile()
    return nc


def _get_nc():
    if "nc" not in _CACHE:
        _CACHE["nc"] = _build()
    return _CACHE["nc"]


def kernel(enc_outputs, x, key_concepts, mask_enc, W_att, b_att, W_hid, b_hid,
           emb_table):
    _install_ntff_shim()
    import ml_dtypes
    from concourse.bass_utils import run_bass_kernel_spmd

    bf = ml_dtypes.bfloat16
    enc_outputs = np.asarray(enc_outputs, dtype=np.float32)
    x = np.asarray(x, dtype=np.float32)
    key_concepts = np.asarray(key_concepts)
    mask_enc = np.asarray(mask_enc)
    W_att = np.asarray(W_att, dtype=np.float32)
    b_att = np.asarray(b_att, dtype=np.float32)
    W_hid = np.asarray(W_hid, dtype=np.float32)
    b_hid = np.asarray(b_hid, dtype=np.float32)
    emb_table = np.asarray(emb_table, dtype=np.float32)

    scale = np.float32(D ** -0.5)
    # host prep: packed enc layout [b, mt, p, c, s_loc], gather, mask encoding
    # encN[b, mt, p, c, s] = enc[b, mt*512 + s, c*128 + p]
    encN = np.ascontiguousarray(
        enc_outputs.reshape(B, NMT, S_MACRO, 4, 128).transpose(0, 1, 4, 3, 2)
    ).astype(bf)
    # xTp[b, p, c, n] = x[b, n, c*128+p]
    xTp = x.reshape(B, N, 8, 128).transpose(0, 3, 2, 1).astype(bf)
    k_emb = emb_table[key_concepts.astype(np.int64)] * scale      # [B,K,D]
    # kembTp[b, p, c, k] = k_emb[b, k, c*128+p]
    kembTp = k_emb.reshape(B, K, 4, 128).transpose(0, 3, 2, 1).astype(bf)
    madd = np.where(mask_enc, np.float32(NEG), np.float32(0.0)).astype(bf)

    batt = b_att.reshape(1, D).astype(bf)
    bhid = np.ascontiguousarray(b_hid.reshape(1, HID))
    Wa = np.ascontiguousarray(
        W_att.reshape(8, 128, 512).transpose(1, 0, 2)).astype(bf)
    Wh = np.ascontiguousarray(
        W_hid.reshape(8, 128, 512).transpose(1, 0, 2)).astype(bf)

    in_maps = []
    for i in range(N_CORES):
        lo, hi = i * BPC, (i + 1) * BPC
        misc = np.concatenate(
            [xTp[lo + b].reshape(128, 8 * N) for b in range(BPC)]
            + [kembTp[lo + b].reshape(128, 4 * K) for b in range(BPC)],
            axis=1)
        rowb = np.concatenate(
            [madd[lo + b].reshape(1, S) for b in range(BPC)] + [batt],
            axis=1)
        in_maps.append({
            "encN": np.ascontiguousarray(encN[lo:hi]),
            "misc": np.ascontiguousarray(misc),
            "rowb": np.ascontiguousarray(rowb),
            "Wa": Wa,
            "Wh": Wh,
            "bhid": bhid,
        })

    nc = _get_nc()
    res = run_bass_kernel_spmd(nc, in_maps, core_ids=list(range(N_CORES)),
                               trace=TRACE)
    LAST_RESULT["exec_time_ns"] = res.exec_time_ns
    LAST_RESULT["mean_exec_time_ns"] = res.mean_exec_time_ns

    # out device layout [BPC, mt, p, st, h] -> [BPC, S, HID]
    outs = []
    for i in range(N_CORES):
        o = np.asarray(res.results[i]["out"])
        if o.dtype != np.float32:
            o = o.astype(np.float32)
        o = o.reshape(BPC, NMT, 128, NST, HID).transpose(0, 1, 3, 2, 4)
        outs.append(o.reshape(BPC, S, HID))
    out = np.concatenate(outs, axis=0)
    ks = np.concatenate([res.results[i]["ks"] for i in range(N_CORES)], axis=0)
    return out, ks


# revision 31
# speedup vs baseline: 1.1915x; 1.0393x over previous
"""Trainium2 Bass kernel for nn_Node_attention_layer (ragged_sequence).

Full-input contract: kernel(**inputs) takes the unsharded inputs and returns
(out [B,S,HID] f32, k_scores [B,S] f32), matching the reference.

Sharding: data-parallel over batch B=16 across 8 NeuronCores (2 samples per
core); Linear weights replicated; emb_table rows gathered host-side (only
K=16 rows per sample are used).

Per-core device program (SPMD, identical program, per-core data):
  proj   = tanh(x @ W_att + b_att)                    [N=64, D=512]
  projW  = proj @ W_hid[D:] + b_hid                   [64, 512]
           (reassociation: dot_x @ Wh2 == scores @ (proj @ Wh2); softmax rows
            sum to 1 so the +b_hid fold is exact)
  per macro tile of 512 s rows:
    lgT = [projT | kembT].T @ encT + ones80 (x) madd  [80, 512]
          (the mask bias rides the matmul accumulation; -1e18 absorbs the
           row's logits in f32 psum, reproducing masked_fill exactly)
    PE-transpose back to [128, 80] tiles; softmax over 64 / over 16
    (pair-batched vector ops); k_scores = 1/sum(exp(klog - kmax))
    out = tanh(encT.T @ Wh1 + scoresT.T @ projW)

Matmul operands are bf16 (host-converted; softmax/psum math stays f32);
enc/out use host-packed per-macro layouts so every DMA moves >=4KB
contiguous per partition.
"""

import sys

if "/opt/trn_rl_repo" not in sys.path:
    sys.path.insert(0, "/opt/trn_rl_repo")

import numpy as np

B, S, D, N, K, V = 16, 2048, 512, 64, 16, 32000
HID = 512
NEG = -1e18
N_CORES = 8
BPC = B // N_CORES  # samples per core
S_MACRO = 512       # s columns per enc staging tile
S_SUB = 128         # s rows per compute tile
NMT = S // S_MACRO      # 4 macro tiles per sample
NST = S_MACRO // S_SUB  # 4 sub tiles per macro
NK = N + K              # 80

OUT_BF16 = False     # stage the tanh output as bf16 (halves write traffic)

TRACE = False       # test.py sets True to collect exec_time_ns
LAST_RESULT = {}    # test.py reads exec_time_ns etc from here

_CACHE = {}


def _install_ntff_shim():
    """Provide antenv.axon_hooks (missing in this image) so that
    run_bass_kernel_spmd(trace=True) can collect NTFF profiles via the
    axon PJRT .so's C ABI."""
    import types
    import ctypes
    import contextlib

    if "antenv.axon_hooks" in sys.modules:
        return
    mod = types.ModuleType("antenv.axon_hooks")
    state = {"hook": None}

    def set_axon_ntff_profile_hook(h):
        state["hook"] = h

    def get_axon_ntff_profile_hook():
        return state["hook"]

    mod.set_axon_ntff_profile_hook = set_axon_ntff_profile_hook
    mod.get_axon_ntff_profile_hook = get_axon_ntff_profile_hook
    sys.modules["antenv.axon_hooks"] = mod
    try:
        import antenv

        antenv.axon_hooks = mod
    except ImportError:
        pass

    so_path = "/opt/axon/libaxon_pjrt.so"
    try:
        lib = ctypes.CDLL(so_path)
        if not hasattr(lib, "axon_start_nrt_profile"):
            return
    except OSError:
        return
    lib.axon_start_nrt_profile.argtypes = [
        ctypes.POINTER(ctypes.c_int64), ctypes.c_size_t]
    lib.axon_start_nrt_profile.restype = ctypes.c_int64
    lib.axon_stop_nrt_profile.argtypes = [ctypes.c_char_p]
    lib.axon_stop_nrt_profile.restype = ctypes.c_int64

    @contextlib.contextmanager
    def _hook(output_dir, device_ids):
        import jax

        jax.devices()
        if device_ids:
            ids = (ctypes.c_int64 * len(device_ids))(*device_ids)
            rc = lib.axon_start_nrt_profile(ids, len(device_ids))
        else:
            rc = lib.axon_start_nrt_profile(None, 0)
        if rc != 0:
            raise RuntimeError(f"axon_start_nrt_profile rc={rc}")
        try:
            yield
        finally:
            n = lib.axon_stop_nrt_profile(str(output_dir).encode())
            print(f"ntff profile: {n} file(s) written to {output_dir}",
                  file=sys.stderr)

    set_axon_ntff_profile_hook(_hook)


def _build():
    import concourse.bacc as bacc
    import concourse.mybir as mybir
    import concourse.tile as tile
    from concourse.masks import make_identity
    import concourse.bass as bass

    f32 = mybir.dt.float32
    bf16 = mybir.dt.bfloat16
    out_dt = bf16 if OUT_BF16 else f32
    AF = mybir.ActivationFunctionType
    AX = mybir.AxisListType
    OP = mybir.AluOpType

    nc = bacc.Bacc("TRN2", target_bir_lowering=False, debug=False,
                   num_devices=N_CORES)

    # enc host-packed bf16: [b, mt, p, c, s_loc] so each macro load is one
    # DMA with 4KB contiguous per partition
    encN_d = nc.dram_tensor("encN", [BPC, NMT, 128, 4, S_MACRO], bf16,
                            kind="ExternalInput").ap()
    # misc: per-partition packed [xT_b0 | xT_b1 | kemb_b0 | kemb_b1]
    misc_d = nc.dram_tensor("misc", [128, 2 * 8 * N + 2 * 4 * K], bf16,
                            kind="ExternalInput").ap()
    # rowb: [madd_b0 | madd_b1 | batt] on one partition row
    rowb_d = nc.dram_tensor("rowb", [1, 2 * S + D], bf16,
                            kind="ExternalInput").ap()
    Wa_d = nc.dram_tensor("Wa", [128, 8, 512], bf16, kind="ExternalInput").ap()
    Wh_d = nc.dram_tensor("Wh", [128, 8, 512], bf16, kind="ExternalInput").ap()
    bhid_d = nc.dram_tensor("bhid", [1, HID], f32, kind="ExternalInput").ap()
    # out host-packed: [b, mt, p, st, h]; host unscrambles
    out_d = nc.dram_tensor("out", [BPC, NMT, 128, NST, HID], out_dt,
                           kind="ExternalOutput").ap()
    ks_d = nc.dram_tensor("ks", [BPC, S], f32, kind="ExternalOutput").ap()

    ks_v = ks_d.rearrange("b (j p) -> b j p", p=128)             # [2,16,128]

    with tile.TileContext(nc) as tc:
        with tc.tile_pool(name="consts", bufs=1) as consts, \
             tc.tile_pool(name="samp", bufs=2) as samp, \
             tc.tile_pool(name="encp", bufs=6) as encp, \
             tc.tile_pool(name="work", bufs=4) as work, \
             tc.tile_pool(name="outp", bufs=2) as outp, \
             tc.tile_pool(name="pslogT", bufs=1, space="PSUM") as pslogT, \
             tc.tile_pool(name="pslg", bufs=1, space="PSUM") as pslg, \
             tc.tile_pool(name="psout", bufs=4, space="PSUM") as psout, \
             tc.tile_pool(name="pst", bufs=2, space="PSUM") as pst:

            ident = consts.tile([128, 128], f32)
            make_identity(nc, ident)
            ident_bf = consts.tile([128, 128], bf16)
            nc.vector.tensor_copy(out=ident_bf, in_=ident)
            ones_bf = consts.tile([1, NK], bf16)
            nc.vector.memset(ones_bf, 1.0)

            # HAM warm-up: ~4us of dense matmuls while the first DMAs
            # stream, so the PE clock is at 2.4GHz when real work arrives
            warm_ps = pst.tile([128, 128], f32, tag="tp")
            for i in range(40):
                nc.tensor.matmul(warm_ps, ident_bf, ident_bf,
                                 start=(i == 0), stop=(i == 39))

            # DMA issue order tuned for the startup critical path:
            # misc (tiny, feeds proj) -> Wa (feeds proj) -> first enc macro
            # -> Wh -> second enc macro
            misc_t = consts.tile([128, 2 * 8 * N + 2 * 4 * K], bf16)
            nc.sync.dma_start(out=misc_t, in_=misc_d)
            Wa_t = consts.tile([128, 8, 512], bf16)
            nc.sync.dma_start(out=Wa_t, in_=Wa_d)
            enc_tiles = {}
            for b, mt in [(0, 0)]:
                enc_t = encp.tile([128, 4, S_MACRO], bf16, tag="enc_t")
                nc.sync.dma_start(out=enc_t, in_=encN_d[b, mt])
                enc_tiles[(b, mt)] = enc_t
            Wh_t = consts.tile([128, 8, 512], bf16)
            nc.sync.dma_start(out=Wh_t, in_=Wh_d)
            for b, mt in [(0, 1)]:
                enc_t = encp.tile([128, 4, S_MACRO], bf16, tag="enc_t")
                nc.sync.dma_start(out=enc_t, in_=encN_d[b, mt])
                enc_tiles[(b, mt)] = enc_t
            rowb_t = consts.tile([1, 2 * S + D], bf16)
            nc.sync.dma_start(out=rowb_t, in_=rowb_d)
            batt_t = rowb_t[:, 2 * S:2 * S + D]
            # b_hid broadcast to 64 partitions (stride-0 partition dim)
            bhid_bc = consts.tile([N, HID], f32)
            bhid_bcast_ap = bass.AP(
                tensor=bhid_d.tensor, offset=bhid_d.offset,
                ap=[[0, N], bhid_d.ap[1]],
            )
            nc.sync.dma_start(out=bhid_bc, in_=bhid_bcast_ap)

            pkT_b = []
            projW_b = []
            maddT_b = []
            kst_b = []
            for b in range(BPC):
                xT_t = misc_t[:, b * 8 * N:(b + 1) * 8 * N].rearrange(
                    "p (c n) -> p c n", c=8)
                kemb_t = misc_t[:, 2 * 8 * N + b * 4 * K:
                                2 * 8 * N + (b + 1) * 4 * K].rearrange(
                    "p (c k) -> p c k", c=4)
                # madd as a free-dim row vector for the PE mask-bias matmul
                maddT = rowb_t[:, b * S:(b + 1) * S]
                maddT_b.append(maddT)

                # proj = tanh(x @ Wa + b_att)  [64, 512]
                proj_ps = pst.tile([N, D], f32, tag="tp")
                for c in range(8):
                    nc.tensor.matmul(proj_ps, xT_t[:, c, :], Wa_t[:, c, :],
                                     start=(c == 0), stop=False)
                nc.tensor.matmul(proj_ps, ones_bf[:, 0:N], batt_t,
                                 start=False, stop=True)
                proj_f = samp.tile([N, D], f32)
                nc.scalar.activation(out=proj_f, in_=proj_ps, func=AF.Tanh)

                # pkT[:, c, 0:64] = projT chunk c;  pkT[:, c, 64:80] = kembT
                pkT = samp.tile([128, 4, NK], bf16)
                for c in range(4):
                    pt_ps = pst.tile([128, N], f32, tag="tp")
                    nc.tensor.transpose(
                        pt_ps, proj_f[:, c * 128:(c + 1) * 128],
                        ident[0:N, 0:N])
                    nc.vector.tensor_copy(out=pkT[:, c, 0:N], in_=pt_ps)
                nc.vector.tensor_copy(out=pkT[:, :, N:NK], in_=kemb_t)
                pkT_b.append(pkT)

                # projW = proj @ Wh2 + b_hid  [64, 512]
                pw_ps = pst.tile([N, HID], f32, tag="tp")
                for c in range(4):
                    nc.tensor.matmul(pw_ps, pkT[:, c, 0:N], Wh_t[:, 4 + c, :],
                                     start=(c == 0), stop=(c == 3))
                # projW duplicated to both partition halves so the scores
                # matmul can use lhsT slices at base partition 0 or 64
                projW_t = samp.tile([2 * N, HID], bf16)
                nc.vector.tensor_add(projW_t[0:N, :], pw_ps, bhid_bc)
                nc.vector.tensor_add(projW_t[N:2 * N, :], pw_ps, bhid_bc)
                projW_b.append(projW_t)

                kst_sb = samp.tile([128, 16], f32, tag="kst")
                kst_b.append(kst_sb)

            for b in range(BPC):
                for mt in range(NMT):
                    if (b, mt) in enc_tiles:
                        continue
                    enc_t = encp.tile([128, 4, S_MACRO], bf16, tag="enc_t")
                    nc.sync.dma_start(out=enc_t, in_=encN_d[b, mt])
                    enc_tiles[(b, mt)] = enc_t

            for b in range(BPC):
                pkT = pkT_b[b]
                projW_t = projW_b[b]
                maddT = maddT_b[b]
                kst_sb = kst_b[b]
                for mt in range(NMT):
                    enc_t = enc_tiles[(b, mt)]

                    # (1) transposed: lgT = [projT|kembT].T @ enc + mask
                    # bias (ones80 outer madd_row rides the accumulation)
                    lgT_ps = pslogT.tile([NK, S_MACRO], f32)
                    nc.tensor.matmul(
                        lgT_ps, ones_bf,
                        maddT[:, mt * S_MACRO:(mt + 1) * S_MACRO],
                        start=True, stop=False)
                    for c in range(4):
                        nc.tensor.matmul(lgT_ps, pkT[:, c, :], enc_t[:, c, :],
                                         start=False, stop=(c == 3))
                    lgT_sb = work.tile([NK, S_MACRO], f32)
                    nc.vector.tensor_copy(out=lgT_sb, in_=lgT_ps)

                    o4_sb = outp.tile([128, NST, HID], out_dt)
                    out_v = out_d[b, mt].rearrange("p (q a) h -> p q a h", q=2)

                    for pr in range(NST // 2):
                        j0 = mt * NST + 2 * pr
                        # transpose two subtiles into one [128, 2, 80] psum
                        lg_ps = pslg.tile([128, 2, NK], f32)
                        for i in range(2):
                            c0 = (2 * pr + i) * S_SUB
                            nc.tensor.matmul(
                                lg_ps[:, i, :], lgT_sb[:, c0:c0 + S_SUB],
                                ident[0:NK, 0:NK], is_transpose=True,
                                start=True, stop=True)

                        # softmax pieces, pair-batched, reading psum direct
                        nmax = work.tile([128, 2], f32)
                        nc.vector.tensor_reduce(
                            out=nmax, in_=lg_ps[:, :, 0:N], axis=AX.X,
                            op=OP.max, negate=True)
                        kmax = work.tile([128, 2], f32)
                        nc.vector.tensor_reduce(
                            out=kmax, in_=lg_ps[:, :, N:NK], axis=AX.X,
                            op=OP.max, negate=True)
                        e_in = work.tile([128, 2, NK], f32)
                        nc.vector.tensor_add(
                            e_in[:, :, 0:N], lg_ps[:, :, 0:N],
                            nmax.broadcast_to([128, 2, N]))
                        nc.vector.tensor_add(
                            e_in[:, :, N:NK], lg_ps[:, :, N:NK],
                            kmax.broadcast_to([128, 2, K]))
                        e_out = work.tile([128, 2, NK], f32)
                        nc.scalar.activation(out=e_out, in_=e_in, func=AF.Exp)
                        nsum = work.tile([128, 2], f32)
                        nc.vector.tensor_reduce(
                            out=nsum, in_=e_out[:, :, 0:N], axis=AX.X,
                            op=OP.add)
                        ksum = work.tile([128, 2], f32)
                        nc.vector.tensor_reduce(
                            out=ksum, in_=e_out[:, :, N:NK], axis=AX.X,
                            op=OP.add)
                        rn = work.tile([128, 2], f32)
                        nc.vector.reciprocal(out=rn, in_=nsum)
                        nc.vector.reciprocal(out=kst_sb[:, j0:j0 + 2],
                                             in_=ksum)
                        ps_sc = work.tile([128, 2, N], f32)
                        nc.vector.tensor_mul(
                            ps_sc, e_out[:, :, 0:N],
                            rn.broadcast_to([128, 2, N]))

                        # one [128,128] transpose covers both subtiles:
                        # rows 0:64 = scoresT of subtile 2pr, 64:128 = 2pr+1
                        scT_ps = pst.tile([128, 128], f32, tag="tp")
                        nc.tensor.transpose(
                            scT_ps, ps_sc.rearrange("p a n -> p (a n)"),
                            ident)
                        scT_sb = work.tile([128, 128], bf16)
                        nc.vector.tensor_copy(out=scT_sb, in_=scT_ps)

                        for i in range(2):
                            st = 2 * pr + i
                            sl = slice(st * S_SUB, (st + 1) * S_SUB)
                            o_ps = psout.tile([128, HID], f32)
                            for c in range(4):
                                nc.tensor.matmul(o_ps, enc_t[:, c, sl],
                                                 Wh_t[:, c, :],
                                                 start=(c == 0), stop=False)
                            nc.tensor.matmul(
                                o_ps, scT_sb[i * N:(i + 1) * N, :],
                                projW_t[i * N:(i + 1) * N, :],
                                start=False, stop=True)
                            nc.scalar.activation(out=o4_sb[:, st, :],
                                                 in_=o_ps, func=AF.Tanh)
                        nc.sync.dma_start(
                            out=out_v[:, pr],
                            in_=o4_sb[:, 2 * pr:2 * pr + 2, :])



                kT_ps = pst.tile([16, 128], f32, tag="tp")
                nc.tensor.transpose(kT_ps, kst_sb, ident)
                kT_sb = samp.tile([16, 128], f32)
                nc.vector.tensor_copy(out=kT_sb, in_=kT_ps)
                nc.sync.dma_start(out=ks_v[b], in_=kT_sb)

    nc.compile()
    return nc


def _get_nc():
    if "nc" not in _CACHE:
        _CACHE["nc"] = _build()
    return _CACHE["nc"]


def kernel(enc_outputs, x, key_concepts, mask_enc, W_att, b_att, W_hid, b_hid,
           emb_table):
    _install_ntff_shim()
    import ml_dtypes
    from concourse.bass_utils import run_bass_kernel_spmd

    bf = ml_dtypes.bfloat16
    enc_outputs = np.asarray(enc_outputs, dtype=np.float32)
    x = np.asarray(x, dtype=np.float32)
    key_concepts = np.asarray(key_concepts)
    mask_enc = np.asarray(mask_enc)
    W_att = np.asarray(W_att, dtype=np.float32)
    b_att = np.asarray(b_att, dtype=np.float32)
    W_hid = np.asarray(W_hid, dtype=np.float32)
    b_hid = np.asarray(b_hid, dtype=np.float32)
    emb_table = np.asarray(emb_table, dtype=np.float32)

    scale = np.float32(D ** -0.5)
    # host prep: packed enc layout [b, mt, p, c, s_loc], gather, mask encoding
    # encN[b, mt, p, c, s] = enc[b, mt*512 + s, c*128 + p]
    encN = np.ascontiguousarray(
        enc_outputs.reshape(B, NMT, S_MACRO, 4, 128).transpose(0, 1, 4, 3, 2)
    ).astype(bf)
    # xTp[b, p, c, n] = x[b, n, c*128+p]
    xTp = x.reshape(B, N, 8, 128).transpose(0, 3, 2, 1).astype(bf)
    k_emb = emb_table[key_concepts.astype(np.int64)] * scale      # [B,K,D]
    # kembTp[b, p, c, k] = k_emb[b, k, c*128+p]
    kembTp = k_emb.reshape(B, K, 4, 128).transpose(0, 3, 2, 1).astype(bf)
    madd = np.where(mask_enc, np.float32(NEG), np.float32(0.0)).astype(bf)

    batt = b_att.reshape(1, D).astype(bf)
    bhid = np.ascontiguousarray(b_hid.reshape(1, HID))
    Wa = np.ascontiguousarray(
        W_att.reshape(8, 128, 512).transpose(1, 0, 2)).astype(bf)
    Wh = np.ascontiguousarray(
        W_hid.reshape(8, 128, 512).transpose(1, 0, 2)).astype(bf)

    in_maps = []
    for i in range(N_CORES):
        lo, hi = i * BPC, (i + 1) * BPC
        misc = np.concatenate(
            [xTp[lo + b].reshape(128, 8 * N) for b in range(BPC)]
            + [kembTp[lo + b].reshape(128, 4 * K) for b in range(BPC)],
            axis=1)
        rowb = np.concatenate(
            [madd[lo + b].reshape(1, S) for b in range(BPC)] + [batt],
            axis=1)
        in_maps.append({
            "encN": np.ascontiguousarray(encN[lo:hi]),
            "misc": np.ascontiguousarray(misc),
            "rowb": np.ascontiguousarray(rowb),
            "Wa": Wa,
            "Wh": Wh,
            "bhid": bhid,
        })

    nc = _get_nc()
    res = run_bass_kernel_spmd(nc, in_maps, core_ids=list(range(N_CORES)),
                               trace=TRACE)
    LAST_RESULT["exec_time_ns"] = res.exec_time_ns
    LAST_RESULT["mean_exec_time_ns"] = res.mean_exec_time_ns

    # out device layout [BPC, mt, p, st, h] -> [BPC, S, HID]
    outs = []
    for i in range(N_CORES):
        o = np.asarray(res.results[i]["out"])
        if o.dtype != np.float32:
            o = o.astype(np.float32)
        o = o.reshape(BPC, NMT, 128, NST, HID).transpose(0, 1, 3, 2, 4)
        outs.append(o.reshape(BPC, S, HID))
    out = np.concatenate(outs, axis=0)
    ks = np.concatenate([res.results[i]["ks"] for i in range(N_CORES)], axis=0)
    return out, ks
